# revision 15
# baseline (speedup 1.0000x reference)
"""Trainium2 Bass kernel for nn_MessageAttentionPassing (v2: t-sharded).

Math (reference):
    h   = x.transpose(0,2,3,1) @ W1 + b1        # [B, N, T, HID]
    mh  = h @ W2[HID:] ; mv = h @ W2[:HID]
    a    = attention[:, 0]                       # [B, N(i), N(j), T]
    upd[b,i,t,c] = asum[b,i,t]*(mv+b2)[b,i,t,c] + sum_j a[b,i,j,t]*mh[b,j,t,c]
    out = upd.transpose(0,3,1,2)                 # [B, COUT, N, T]

Key observations exploited here:
  1. Everything is independent across t -> shard 8 cores = (batch b) x
     (t-quarter q of 6 steps).  Each core computes ALL 128 nodes for its
     6 t's; no replication of x across node-shards (vs the old i-sharded
     layout this is ~2x less DMA) and no np.roll tricks.
  2. The chain is linear in x, so W1 folds into W2 on the host:
         mh_t  = x1_t^T @ [W1@W2h ; b1@W2h]          (ones row -> bias)
         mvb_t = x1_t^T @ [W1@W2v ; b1@W2v + b2]
     which kills the separate h/mv/mh matmuls entirely: ONE 128-col
     matmul per t (stationary x1_t [65,128], moving Wcat [65,128]).
  3. The einsum contracts j on partitions: stationary a_tT [j,i] [128,128],
     moving mh1_t [j, 64+1] where col 64 is constant 1.0 -> out col 64 is
     asum_t for free.
  4. Final combine is ONE fused op per t:
         out_t = (mvb_t * asum_t) + P_t          (scalar_tensor_tensor)
     reading both PSUM banks directly (scalar AP may live in PSUM).
  5. fp16 everywhere (tolerance 2e-2; measured end-to-end err ~4e-4):
     4x fewer PE cycles than fp32 and half the DMA bytes.

Per core per rep: 12 matmuls, 3 paired copies (ACT), 6 fused combines
(split DVE/Pool), 2 input DMAs, 1 output DMA.

The `reps`/`ablate` knobs exist only for benchmarking (the per-rep loop
amortizes the ~70 ms axon dispatch overhead; ablate drops stages).
"""

import os
import sys
import numpy as np

if "/opt/trn_rl_repo" not in sys.path:
    sys.path.insert(0, "/opt/trn_rl_repo")

B, CIN, N, T, COUT, HID = 2, 64, 128, 24, 64, 64
TQ = T // 4          # t-steps per core: 6
FX = TQ * N          # x / att free cols: 768
C1 = CIN + 1         # cin + ones row: 65
MH1 = COUT + 1       # mh cols + ones col: 65
FO = TQ * COUT       # out free cols: 384

_PROGRAM = None      # compiled program cache — compile once per process

_STAGE_OUT = {       # benchmark stage -> tile it writes (for ablation stubs)
    "xdma": ["xw"], "attdma": ["attT"], "mm1": ["ps1"], "cp": ["cm"],
    "mm2": ["ps2"], "fin": ["outT"],
}


def _build_program(reps: int = 1, ablate: frozenset = frozenset()):
    import concourse.bacc as bacc
    from concourse import mybir, tile

    f16 = mybir.dt.float16
    f32 = mybir.dt.float32

    nc = bacc.Bacc(
        "TRN2",
        target_bir_lowering=False,
        debug=False,
        enable_asserts=False,
        num_devices=8,
    )

    dram = {
        "xw": nc.dram_tensor("xw", [C1, FX + 128], f16, kind="ExternalInput"),
        "att": nc.dram_tensor("att", [N, FX], f16, kind="ExternalInput"),
        "out": nc.dram_tensor("out", [N, FO], f16, kind="ExternalOutput"),
    }

    with tile.TileContext(nc) as tc:
        with (
            tc.tile_pool(name="const", bufs=1) as cpool,
            tc.tile_pool(name="ps", bufs=1, space="PSUM") as pspool,
        ):
            tl = {}
            tl["xw"] = cpool.tile([C1, FX + 128], f16, name="xw")
            tl["attT"] = cpool.tile([N, FX], f16, name="attT")
            # cm: per-t 129-col group [one | mh (64) | mvb (64)]
            tl["cm"] = cpool.tile([N, TQ * 129], f16, name="cm")
            tl["outT"] = cpool.tile([N, FO], f16, name="outT")
            # ones columns (col 0 of each 129-col group) written once here;
            # the per-rep copies only touch cols 1:129 of each group.
            nc.vector.memset(tl["cm"][:], 1.0)
            # one PSUM bank ([128,512] f32) per t-pair and per matmul stage:
            # separate tiles keep the framework's WAR tracking per-pair so
            # mm1 of pair g+1 never waits on the cp read of pair g.
            for g in range(TQ // 2):
                tl[f"ps1{g}"] = pspool.tile([N, 512], f32, name=f"ps1{g}")
                tl[f"ps2{g}"] = pspool.tile([N, 512], f32, name=f"ps2{g}")

            for stage in ablate:
                for nm in _STAGE_OUT.get(stage, ()):
                    if tl[nm].space_name == "PSUM":
                        continue
                    nc.vector.memset(tl[nm][:], 0.0)

            for _rep in range(reps):
                _rep_body(nc, tl, dram, mybir, ablate)

    nc.compile()
    return nc


def _rep_body(nc, tl, dram, mybir, ablate=frozenset()):
    xw, attT, cm, outT = tl["xw"], tl["attT"], tl["cm"], tl["outT"]
    mul = mybir.AluOpType.mult
    add = mybir.AluOpType.add

    # ---- input DMAs. Both on the SP ring: HWDGE generation is a single
    # shared resource so a second ring buys nothing, and SP's DGE delay is
    # the shortest. xw first (it gates mm1, the head of the chain). ----
    if "xdma" not in ablate:
        nc.sync.dma_start(xw[:], dram["xw"][:])
    if "attdma" not in ablate:
        nc.sync.dma_start(attT[:], dram["att"][:])

    wcat = xw[:, FX:FX + 128]                       # moving [65, 128]
    cmv = cm[:].rearrange("p (t c) -> p t c", c=129)

    def mm1(t):
        # ps1_t = [mh_t | mvb_t]: stationary x1_t [65,128], moving Wcat.
        ps = tl[f"ps1{t // 2}"]
        nc.tensor.matmul(
            ps[:, (t % 2) * 128:(t % 2) * 128 + 128],
            xw[:, t * 128:(t + 1) * 128], wcat, start=True, stop=True,
        )

    def cp_mh(g):
        # paired PSUM->SBUF fp16 copy of mh for t=2g, 2g+1 (ACT).
        # mh alone unblocks mm2; the mvb copy is deferred to cp_mvb.
        ps = tl[f"ps1{g}"]
        psv = ps[:, 0:256].rearrange("p (t c) -> p t c", c=128)
        nc.scalar.copy(cmv[:, 2 * g:2 * g + 2, 1:MH1], psv[:, :, 0:COUT])

    def cp_mvb(g):
        # mvb drains on whichever of DVE/ACT has slack (DVE is idle until
        # the first fin; ACT is the serial cp_mh chain)
        ps = tl[f"ps1{g}"]
        psv = ps[:, 0:256].rearrange("p (t c) -> p t c", c=128)
        eng = nc.scalar if g == 1 else nc.vector
        if eng is nc.scalar:
            eng.copy(cmv[:, 2 * g:2 * g + 2, MH1:129], psv[:, :, COUT:128])
        else:
            eng.tensor_copy(cmv[:, 2 * g:2 * g + 2, MH1:129],
                            psv[:, :, COUT:128])

    def mm2(t):
        # ps2_t = [asum_t | P_t]: stationary a_tT [128,128],
        # moving [one | mh_t] (contiguous 65 cols of cm).
        ps = tl[f"ps2{t // 2}"]
        nc.tensor.matmul(
            ps[:, (t % 2) * MH1:(t % 2) * MH1 + MH1],
            attT[:, t * 128:(t + 1) * 128],
            cm[:, t * 129:t * 129 + MH1], start=True, stop=True,
        )

    def fin(t):
        # out_t = (mvb_t * asum_t) + P_t; only ps2 is a PSUM tensor input
        # (DVE only: GPSIMD cannot access PSUM, ACT has no fused op)
        ps = tl[f"ps2{t // 2}"]
        o = (t % 2) * MH1
        nc.vector.scalar_tensor_tensor(
            outT[:, t * COUT:(t + 1) * COUT],
            cm[:, t * 129 + MH1:t * 129 + 129],
            ps[:, o:o + 1],
            ps[:, o + 1:o + MH1],
            op0=mul, op1=add,
        )

    if "mm1" not in ablate:
        mm1(0), mm1(1), mm1(2), mm1(3), mm1(4), mm1(5)
    for g in range(TQ // 2):
        if "cp" not in ablate:
            cp_mh(g)
            cp_mvb(g)
        if "mm2" not in ablate:
            mm2(2 * g), mm2(2 * g + 1)
    for t in range(TQ):
        if "fin" not in ablate:
            fin(t)
        # first output half leaves while t=4,5 still compute
        if t == 3 and "outdma" not in ablate:
            nc.sync.dma_start(dram["out"][:, 0:4 * COUT], outT[:, 0:4 * COUT])
    if "outdma" not in ablate:
        nc.scalar.dma_start(dram["out"][:, 4 * COUT:FO], outT[:, 4 * COUT:FO])


def _get_program():
    global _PROGRAM
    if _PROGRAM is None:
        _PROGRAM = _build_program()
    return _PROGRAM


def _make_in_maps(x, attention, W1, b1, W2, b2):
    x = np.asarray(x, dtype=np.float64)
    attention = np.asarray(attention, dtype=np.float64)
    W1 = np.asarray(W1, dtype=np.float64)
    b1 = np.asarray(b1, dtype=np.float64)
    W2 = np.asarray(W2, dtype=np.float64)
    b2 = np.asarray(b2, dtype=np.float64)

    Wh = W1 @ W2[HID:]
    bh = b1 @ W2[HID:]
    Wv = W1 @ W2[:HID]
    bv = b1 @ W2[:HID] + b2
    wcat = np.concatenate(
        [np.vstack([Wh, bh[None]]), np.vstack([Wv, bv[None]])], axis=1
    ).astype(np.float16)                              # [65, 128]

    in_maps = []
    for k in range(8):
        b, q = k // 4, k % 4
        t0 = q * TQ
        # x1 [65, (t,i)]: rows 0:64 = x[b,:,i,t] laid out t-major, row 64 = 1
        xs = x[b][:, :, t0:t0 + TQ].transpose(0, 2, 1).reshape(CIN, FX)
        x1 = np.concatenate([xs, np.ones((1, FX))], axis=0)
        xwm = np.concatenate([x1, wcat], axis=1).astype(np.float16)
        # attT [j, (t,i)] so each [128,128] t-slice is a_t^T (j rows, i cols)
        att_c = np.ascontiguousarray(
            attention[b, 0][:, :, t0:t0 + TQ].transpose(1, 2, 0).reshape(N, FX)
        ).astype(np.float16)
        in_maps.append({"xw": xwm, "att": att_c})
    return in_maps


def run(inputs: dict, trace: bool = False):
    """Compile (cached), shard, run on 8 cores; returns (full_out, results)."""
    from concourse import bass_utils

    nc = _get_program()
    in_maps = _make_in_maps(**inputs)
    res = bass_utils.run_bass_kernel_spmd(
        nc, in_maps, core_ids=list(range(8)), trace=trace,
    )
    full = np.empty((B, COUT, N, T), dtype=np.float32)
    for k in range(8):
        b, q = k // 4, k % 4
        t0 = q * TQ
        o = res.results[k]["out"].astype(np.float32).reshape(N, TQ, COUT)
        full[b, :, :, t0:t0 + TQ] = o.transpose(2, 0, 1)
    return full, res


def kernel(**inputs) -> np.ndarray:
    full, _ = run(inputs, trace=False)
    return full
